# revision 1
# baseline (speedup 1.0000x reference)
"""Multi-head attention (B=2, S=2048, D=1024, H=16) on 8 Trainium2 cores.

Sharding: tensor-parallel over heads (4 groups of 4 heads) x data-parallel
over batch (2). Core c handles batch c//4, head group c%4. The output
projection is column-sharded: after an AllGather of ctx over the 4-core
group, core c computes out[:, 256g:256(g+1)] for all sq of its batch.

Per-core plan (activations kept feature-on-partition, i.e. transposed):
  qT/kT = (w[:,local].T @ x.T)      [256, 2048]
  V     = x @ w_v[:,local] (natural), stored per 128-row sk chunk with an
          extra ones column per head -> the PV matmul also accumulates the
          softmax denominators for free (row 64 of ctx')
  scores_T[sk, sq] = kT_blk.T @ qT  (2 heads packed in PE rows 0-63 / 64-127)
  causal: blocks above the diagonal skipped, additive tril tile on diagonal
  exp on ScalarE (scale=1/sqrt(dk) folded in; no max subtraction: scores are
  ~N(0,1) so exp cannot overflow, matching softmax exactly in exact math)
  ctx'_T[65, sq] += V'_chunk.T @ exp_T_chunk
  ctx_T = ctx'_T[:64] * broadcast(1/denom)  (broadcast via selector matmul)
  AllGather ctx_T over the group; out_T[256, S] = w_o[:, local].T @ ctx_full
Output per core: [256, 2048] fp32 = out[b]^T slice; host transposes/concats.
"""
import os
import numpy as np

import concourse.bass as bass
import concourse.mybir as mybir
import concourse.tile as tile
import bass_rust as _bass_rust
from concourse.bass_utils import run_bass_kernel_spmd

dt = mybir.dt
AF = mybir.ActivationFunctionType
ALU = mybir.AluOpType

B, S, D, H = 2, 2048, 1024, 16
DK = D // H          # 64
HL = 4               # heads per core
DL = HL * DK         # 256 local head dims
NCORE = 8
GROUPS = [[0, 1, 2, 3], [4, 5, 6, 7]]
SQG = 512            # sq group width (one PSUM bank)
NSQG = S // SQG      # 4
NSK = S // 128       # 16 sk blocks
KCH = D // 128       # 8 contraction chunks for projections
SCALE = 1.0 / float(np.sqrt(np.float32(DK)))
NEG = -1e9

DTNAME = os.environ.get("KERNEL_DT", "f32r")
_DT_NP = {"f16": np.float16, "f32r": np.float32, "f32": np.float32}
_DT_MY = {"f16": dt.float16, "f32r": dt.float32r, "f32": dt.float32}

LAST_RESULT = None   # BassKernelResults of the most recent run (profiling)
_CACHE = {}          # (dtname, causal) -> built Bass


def _split_multiwait(nc):
    """This walrus supports one sync-wait per instruction; Tile emits several.
    Hoist all but the last wait of each instruction onto single-wait NOPs
    placed immediately before it on the same engine."""
    for bbw in nc.bb_map.values():
        insts = bbw.bb.instructions
        out = []
        for inst in insts:
            si = inst.sync_info
            waits = list(si.on_wait or []) if si is not None else []
            if len(waits) > 1:
                for w in waits[:-1]:
                    nop = _bass_rust.InstNoOp(
                        name=nc.get_next_instruction_name(), ins=[], outs=[])
                    nop.engine = inst.engine
                    nop.bass_nofuse = True
                    nop.sync_info = mybir.SyncInfo(on_wait=[w], on_update=[])
                    nc.register_instruction(nop)
                    out.append(nop)
                inst.sync_info = mybir.SyncInfo(
                    on_wait=[waits[-1]], on_update=list(si.on_update or []))
            out.append(inst)
        insts[:] = out


def _build(dtname: str, causal: bool):
    DT = _DT_MY[dtname]
    nc = bass.Bass(num_devices=NCORE)

    xq = nc.declare_dram_parameter("xq", [D, S], DT, isOutput=False)
    xk = nc.declare_dram_parameter("xk", [D, S], DT, isOutput=False)
    xv = nc.declare_dram_parameter("xv", [D, S], DT, isOutput=False)
    wq = nc.declare_dram_parameter("wq", [D, DL], DT, isOutput=False)
    wk = nc.declare_dram_parameter("wk", [D, DL], DT, isOutput=False)
    wv = nc.declare_dram_parameter("wv", [D, DL], DT, isOutput=False)
    wo = nc.declare_dram_parameter("wo", [DL, D], dt.float16, isOutput=False)
    mask_t = nc.declare_dram_parameter("mask_t", [128, 128], dt.float32,
                                       isOutput=False)
    ones_c = nc.declare_dram_parameter("ones_c", [128, 64], DT, isOutput=False)
    ones_r = nc.declare_dram_parameter("ones_r", [1, 64], DT, isOutput=False)
    out = nc.declare_dram_parameter("out", [2, 128, S], dt.float16, isOutput=True)
    debug = os.environ.get("KERNEL_DEBUG", "0") == "1"
    if debug:
        d_qT = nc.declare_dram_parameter("d_qT", [2, 128, S], DT, isOutput=True)
        d_kT = nc.declare_dram_parameter("d_kT", [2, 128, S], DT, isOutput=True)
        d_Vp = nc.declare_dram_parameter("d_Vp", [128, NSK, 65 * HL], DT, isOutput=True)
        d_ctx = nc.declare_dram_parameter("d_ctx", [128, 2, S], dt.float16, isOutput=True)

    with tile.TileContext(nc) as tc:
        with (
            tc.tile_pool(name="wpool", bufs=1) as wpool,
            tc.tile_pool(name="xpool", bufs=3) as xpool,
            tc.tile_pool(name="apool", bufs=1) as apool,
            tc.tile_pool(name="epool", bufs=3) as epool,
            tc.tile_pool(name="opool", bufs=2) as opool,
            tc.tile_pool(name="psA", bufs=2, space="PSUM") as psA,
            tc.tile_pool(name="psB", bufs=2, space="PSUM") as psB,
            tc.tile_pool(name="dram", bufs=1, space="DRAM") as drp,
        ):
            # ---- resident weights / constants ----
            wq_sb = wpool.tile([128, KCH, DL], DT, tag="wq")
            wk_sb = wpool.tile([128, KCH, DL], DT, tag="wk")
            wv_sb = wpool.tile([128, KCH, DL], DT, tag="wv")
            wo_sb = wpool.tile([128, 2, D], dt.float16, tag="wo")
            nc.sync.dma_start(wq_sb[:], wq.rearrange("(c p) m -> p c m", p=128))
            nc.sync.dma_start(wk_sb[:], wk.rearrange("(c p) m -> p c m", p=128))
            nc.sync.dma_start(wv_sb[:], wv.rearrange("(c p) m -> p c m", p=128))
            nc.sync.dma_start(wo_sb[:], wo.rearrange("(c p) m -> p c m", p=128))
            mask_sb = wpool.tile([128, 128], dt.float32, tag="mask")
            nc.sync.dma_start(mask_sb[:], mask_t[:])
            ones64 = wpool.tile([1, 64], DT, tag="ones64")
            nc.sync.dma_start(ones64[:], ones_r[:])

            # ---- persistent activations ----
            qT = [apool.tile([128, S], DT, tag=f"qT{hp}", name=f"qT{hp}") for hp in range(2)]
            kT = [apool.tile([128, S], DT, tag=f"kT{hp}", name=f"kT{hp}") for hp in range(2)]
            Vp = apool.tile([128, NSK, 65 * HL], DT, tag="Vp")
            nc.sync.dma_start(
                Vp.rearrange("p i (h e) -> p i h e", e=65)[:, :, :, 64:65],
                ones_c.rearrange("p (i h one) -> p i h one", h=HL, one=1))
            ctx_sb = apool.tile([128, 2, S], dt.float16, tag="ctx")

            # ---- fused pipeline over sq groups ----
            # per sg: project q/k/v columns for sg, run attention jg=sg
            # (its keys only need kT/V chunks <= sg), AllGather its ctx
            # columns, and emit the out-projection for sg-1 behind it.
            agos = []

            def proj_sg(sg):
                for tname, xin, w_sb, dst in (
                        ("q", xq, wq_sb, qT), ("k", xk, wk_sb, kT)):
                    pss = [psA.tile([128, SQG], dt.float32, tag=f"pj{cc}",
                                    name=f"pj{cc}", bufs=1)
                           for cc in range(2)]
                    for kk in range(KCH):
                        xt = xpool.tile([128, SQG], DT, tag=f"x{tname}")
                        nc.sync.dma_start(
                            xt[:], xin[128 * kk:128 * (kk + 1),
                                       SQG * sg:SQG * (sg + 1)])
                        for cc in range(2):
                            nc.tensor.matmul(
                                pss[cc][:],
                                lhsT=w_sb[:, kk, 128 * cc:128 * (cc + 1)],
                                rhs=xt[:],
                                start=(kk == 0), stop=(kk == KCH - 1))
                    for cc in range(2):
                        nc.vector.tensor_copy(
                            dst[cc][:, SQG * sg:SQG * (sg + 1)], pss[cc][:])
                xvt = xpool.tile([128, KCH, SQG], DT, tag="xv", bufs=2)
                for kk in range(KCH):
                    nc.sync.dma_start(
                        xvt[:, kk, :], xv[128 * kk:128 * (kk + 1),
                                          SQG * sg:SQG * (sg + 1)])
                for half in range(2):
                    psv = [psA.tile([128, SQG], dt.float32, tag=f"pj{j}",
                                    name=f"pv{j}", bufs=1)
                           for j in range(2)]
                    for kk in range(KCH):
                        for j in range(2):
                            sc = 2 * half + j
                            nc.tensor.matmul(
                                psv[j][:, :DL],
                                lhsT=xvt[:, kk, 128 * sc:128 * (sc + 1)],
                                rhs=wv_sb[:, kk, :],
                                start=(kk == 0), stop=(kk == KCH - 1))
                    for j in range(2):
                        sc = 2 * half + j
                        i = 4 * sg + sc
                        vdst = Vp[:, i].rearrange("p (h e) -> p h e", e=65)
                        nc.vector.tensor_copy(
                            vdst[:, :, :64],
                            psv[j][:, :DL]
                            .rearrange("p (h e) -> p h e", e=64))

            def attn_jg(jg):
                for hp in range(2):
                    nsk = 4 * jg + 4 if causal else NSK
                    ctx_ps = [psB.tile([65, SQG], dt.float32, tag=f"ctx{m}",
                                       name=f"ctx{m}", bufs=1)
                              for m in range(2)]
                    for i in range(nsk):
                        col0 = 128 * max(0, i - 4 * jg) if causal else 0
                        ets = []
                        for m in range(2):
                            sps = psA.tile([128, SQG], dt.float32,
                                           tag=f"sc{m}", name=f"sps{m}")
                            nc.tensor.matmul(
                                sps[:, col0:SQG],
                                lhsT=kT[hp][64 * m:64 * m + 64,
                                            128 * i:128 * (i + 1)],
                                rhs=qT[hp][64 * m:64 * m + 64,
                                           SQG * jg + col0:SQG * (jg + 1)],
                                start=True, stop=True)
                            if causal and i >= 4 * jg:
                                nc.vector.tensor_tensor(
                                    sps[:, col0:col0 + 128],
                                    sps[:, col0:col0 + 128],
                                    mask_sb[:], ALU.add)
                            et = epool.tile([128, SQG], DT, tag=f"exp{m}")
                            nc.scalar.activation(
                                et[:, col0:SQG], sps[:, col0:SQG],
                                AF.Exp, scale=SCALE)
                            ets.append(et)
                        for m in range(2):
                            hl = 2 * hp + m
                            nc.tensor.matmul(
                                ctx_ps[m][:, col0:SQG],
                                lhsT=Vp[:, i, 65 * hl:65 * hl + 65],
                                rhs=ets[m][:, col0:SQG],
                                start=(i == 0), stop=(i == nsk - 1))
                    for m in range(2):
                        recip = opool.tile([1, SQG], dt.float32,
                                           tag=f"recip{m}", name=f"recip{m}")
                        nc.vector.reciprocal(recip[:], ctx_ps[m][64:65, :])
                        recip_dt = opool.tile([1, SQG], DT,
                                              tag=f"recipdt{m}",
                                              name=f"recipdt{m}")
                        nc.vector.tensor_copy(recip_dt[:], recip[:])
                        bc = psA.tile([128, SQG], dt.float32, tag=f"pj{m}",
                                      name=f"bc{m}", bufs=1)
                        nc.tensor.matmul(bc[0:64, :], lhsT=ones64[:],
                                         rhs=recip_dt[:],
                                         start=True, stop=True)
                        bc_sb = opool.tile([64, SQG], dt.float32,
                                           tag=f"bcsb{m}", name=f"bc_sb{m}")
                        nc.vector.tensor_copy(bc_sb[:], bc[0:64, :])
                        nc.vector.tensor_tensor(
                            ctx_sb[64 * m:64 * m + 64, hp,
                                   SQG * jg:SQG * (jg + 1)],
                            ctx_ps[m][0:64, :],
                            bc_sb[:], ALU.mult)

            def outproj_rs_sg(sg):
                # partial out for ALL 1024 ocols from the local 256 ctx dims,
                # then ReduceScatter(add) over the group: rank r receives the
                # summed ocol quarter r for this sq slice = its final output.
                par_sb = opool.tile([128, KCH, SQG], dt.float16, tag="par",
                                    name="par", bufs=2)
                for oc in range(KCH):
                    pso = psA.tile([128, SQG], dt.float32, tag=f"pj{oc % 2}",
                                   name=f"pso{oc % 2}", bufs=1)
                    for kc in range(2):
                        nc.tensor.matmul(
                            pso[:],
                            lhsT=wo_sb[:, kc, 128 * oc:128 * (oc + 1)],
                            rhs=ctx_sb[:, kc, SQG * sg:SQG * (sg + 1)],
                            start=(kc == 0), stop=(kc == 1))
                    nc.vector.tensor_copy(par_sb[:, oc, :], pso[:])
                part = drp.tile([KCH, 128, SQG], dt.float16, name=f"part{sg}")
                for oc in range(KCH):
                    nc.sync.dma_start(part[oc], par_sb[:, oc, :])
                rsout = drp.tile([2, 128, SQG], dt.float16, name=f"rso{sg}")
                nc.gpsimd.collective_compute(
                    "ReduceScatter", ALU.add, replica_groups=GROUPS,
                    ins=[part.opt()], outs=[rsout.opt()])
                nc.sync.dma_start(out[:, :, SQG * sg:SQG * (sg + 1)],
                                  rsout[:])

            for sg in range(NSQG):
                proj_sg(sg)
                attn_jg(sg)
                outproj_rs_sg(sg)

            if debug:
                for hp in range(2):
                    nc.sync.dma_start(d_qT[hp], qT[hp][:])
                    nc.sync.dma_start(d_kT[hp], kT[hp][:])
                    nc.sync.dma_start(d_ctx[:, hp, :], ctx_sb[:, hp, :])
                nc.sync.dma_start(d_Vp[:], Vp[:])

    _split_multiwait(nc)
    return nc


def _mask_kind(mask: np.ndarray) -> bool:
    """True if causal (tril), False if all-ones; raises otherwise."""
    m = np.asarray(mask).reshape(S, S)
    if np.array_equal((m != 0).astype(np.int8), np.tril(np.ones((S, S), np.int8))):
        return True
    if np.all(m != 0):
        return False
    raise NotImplementedError("unsupported mask pattern")


def kernel(q, k, v, mask, w_q, b_q, w_k, b_k, w_v, b_v, w_o, b_o):
    global LAST_RESULT
    assert not np.any(b_q) and not np.any(b_k) and not np.any(b_v) \
        and not np.any(b_o), "nonzero biases not supported"
    dtname = DTNAME
    npdt = _DT_NP[dtname]
    causal = _mask_kind(mask)

    key = (dtname, causal)
    if key not in _CACHE:
        _CACHE[key] = _build(dtname, causal)
    nc = _CACHE[key]

    q = np.asarray(q, np.float32)
    k = np.asarray(k, np.float32)
    v = np.asarray(v, np.float32)
    # transposed per-batch activations
    xqs = [np.ascontiguousarray(q[b].T).astype(npdt) for b in range(B)]
    xks = [np.ascontiguousarray(k[b].T).astype(npdt) for b in range(B)]
    xvs = [np.ascontiguousarray(v[b].T).astype(npdt) for b in range(B)]
    wqs = [np.ascontiguousarray(np.asarray(w_q, np.float32)[:, DL * g:DL * (g + 1)]).astype(npdt) for g in range(4)]
    wks = [np.ascontiguousarray(np.asarray(w_k, np.float32)[:, DL * g:DL * (g + 1)]).astype(npdt) for g in range(4)]
    wvs = [np.ascontiguousarray(np.asarray(w_v, np.float32)[:, DL * g:DL * (g + 1)]).astype(npdt) for g in range(4)]
    wos = [np.ascontiguousarray(np.asarray(w_o, np.float32)[DL * g:DL * (g + 1), :]).astype(np.float16) for g in range(4)]
    # additive tril tile in scores_T layout: (sk_row p, sq_col f) valid iff p<=f
    onc = np.ones((128, 64), npdt)
    onr = np.ones((1, 64), npdt)
    mt = np.where(np.arange(128)[:, None] <= np.arange(128)[None, :],
                  np.float32(0), np.float32(NEG))

    in_maps = []
    for c in range(NCORE):
        b, g = c // 4, c % 4
        in_maps.append({
            "xq": xqs[b], "xk": xks[b], "xv": xvs[b],
            "wq": wqs[g], "wk": wks[g], "wv": wvs[g], "wo": wos[g],
            "mask_t": mt, "ones_c": onc, "ones_r": onr,
        })
    res = run_bass_kernel_spmd(nc, in_maps, core_ids=list(range(NCORE)))
    LAST_RESULT = res

    outf = np.empty((B, S, D), np.float32)
    for c in range(NCORE):
        b, g = c // 4, c % 4
        o = res.results[c]["out"].reshape(DL, S).astype(np.float32)
        outf[b, :, DL * g:DL * (g + 1)] = o.T
    return outf

